# revision 3
# baseline (speedup 1.0000x reference)
"""ContrastiveLoss Trainium2 kernel (8 NeuronCores, SPMD).

Math: reference computes, over p = predict.reshape(B, D):
    d2_ij  = max(||p_i||^2 + ||p_j||^2 - 2 p_i.p_j, 0) / D
    lab_ij = [g_i == g_j]
    term   = lab*d2 + (1-lab)*relu(M - d2),  M = 2
    loss   = 2 * sum_{j>=i} term / (B*(B-1))

Device formulation (full matrix, diagonal self-cancelling):
  q = bf16(sqrt(2)*p); stil_i = (q_i.q_i)/2  (host fp32)
  PSUM_A = q_i.q_j - (stil_j - C)            (bf16 GEMM K=2048 + K=2 feature GEMM)
  d2     = relu(-PSUM_A/D + (stil_i + C)/D)  == ||p^_i - p^_j||^2/D  >= 0 exactly
  t      = relu(M - d2)                      (scalar engine, accum_out -> col sums)
  PSUM_B = dg2 - 2/LAM  where dg2 = sum_digit (d_i - d_j)^2  (exact bf16 GEMM K=10)
  w      = relu((d2 - t) + 2 - LAM*dg2)      (vector engine, accum_out -> col sums)
  sum over full matrix:  S = sum t + sum w  satisfies
  loss   = (S - 2*N_label) / (B*(B-1)),  N_label = sum_c count_c^2 (ordered, incl diag)
Each core owns 1024 rows x all 8192 cols; host sums 8x2 [128,128] accumulators.
"""

import os
import math
from contextlib import ExitStack

import numpy as np
import ml_dtypes

BF16 = ml_dtypes.bfloat16

B = 8192
DFEAT = 2048
NCORES = 8
P = 128
WINW = 512
MARGIN = 2.0
C_OFF = 2048.0
LAM = 64.0

LAST_EXEC_NS = None
LAST_RESULTS = None


def tri_entries(ncores=NCORES, nblks=16):
    """Per-core list of (row_block, col_block) in 512-units: 2 diagonal entries
    first, then 15 plain upper-triangle entries. Uniform length for every core."""
    per_core = []
    for k in range(ncores):
        a, bb = k, nblks - 1 - k
        ents = [(a, a), (bb, bb)]
        ents += [(a, c) for c in range(a + 1, nblks)]
        ents += [(bb, c) for c in range(bb + 1, nblks)]
        assert len(ents) == nblks + 1
        per_core.append(ents)
    return per_core


def build_tri_nc(b=B, dfeat=DFEAT, ncores=NCORES, use_fp8=False, reps=1):
    """Triangle variant: 17 host-packed (lhs, rhs) stream entries per core,
    4 row-tiles each; entries 0,1 are diagonal blocks masked to j > i.
    use_fp8: main GEMM in float8e4 with DoubleRow (2 K-chunks per matmul).
    reps: repeat the whole body (timing builds; outputs are overwritten
    identically each rep)."""
    import concourse.bacc as bacc
    import concourse.mybir as mybir
    from concourse.tile import TileContext

    fp32 = mybir.dt.float32
    bf16 = mybir.dt.bfloat16
    qdt = mybir.dt.float8e4 if use_fp8 else bf16
    Act = mybir.ActivationFunctionType
    Alu = mybir.AluOpType

    kch = dfeat // P               # contraction chunks
    nent = (b // WINW) + 1         # 17 stream entries
    rt_n = WINW // P               # 4 row tiles per entry
    nblk = nent * rt_n             # 68 accumulator columns

    nc = bacc.Bacc()
    lhs_s = nc.declare_dram_parameter("lhs_s", [nent, dfeat, WINW], qdt, isOutput=False)
    rhs_s = nc.declare_dram_parameter("rhs_s", [nent, dfeat, WINW], qdt, isOutput=False)
    sqf_s = nc.declare_dram_parameter("sqf_s", [nent, 2, WINW], bf16, isOutput=False)
    labr_s = nc.declare_dram_parameter("labr_s", [nent, 10, WINW], bf16, isOutput=False)
    labl_s = nc.declare_dram_parameter("labl_s", [nent, 10, WINW], bf16, isOutput=False)
    bias_s = nc.declare_dram_parameter("bias_s", [nent, P, rt_n], fp32, isOutput=False)
    acct_d = nc.declare_dram_parameter("acc_t", [P, nblk], fp32, isOutput=True)
    accw_d = nc.declare_dram_parameter("acc_w", [P, nblk], fp32, isOutput=True)

    with TileContext(nc) as tc, ExitStack() as ctx:
        const = ctx.enter_context(tc.tile_pool(name="const", bufs=1))
        str_pool = ctx.enter_context(tc.tile_pool(name="streams", bufs=3))
        sc_pool = ctx.enter_context(tc.tile_pool(name="scratch", bufs=3))
        psA_pool = ctx.enter_context(tc.tile_pool(name="psA", bufs=2, space="PSUM"))
        psB_pool = ctx.enter_context(tc.tile_pool(name="psB", bufs=2, space="PSUM"))
        acc_pool = ctx.enter_context(tc.tile_pool(name="acc", bufs=1))

        ones2 = const.tile([2, P], bf16)
        nc.any.memset(ones2[:], 1.0)
        mbias = const.tile([P, 1], fp32)
        nc.any.memset(mbias[:], MARGIN)
        acct_t = acc_pool.tile([P, nblk], fp32)
        accw_t = acc_pool.tile([P, nblk], fp32)

        lhs_r = lhs_s.rearrange("e (c p) n -> e p c n", p=P)
        rhs_r = rhs_s.rearrange("e (c p) n -> e p c n", p=P)

        for e in [e for _ in range(reps) for e in range(nent)]:
            lhs_t = str_pool.tile([P, kch, WINW], qdt, tag="lhs")
            nc.sync.dma_start(out=lhs_t[:], in_=lhs_r[e])
            rhs_t = str_pool.tile([P, kch, WINW], qdt, tag="rhs")
            nc.sync.dma_start(out=rhs_t[:], in_=rhs_r[e])
            sqf_t = str_pool.tile([2, WINW], bf16, tag="sqf")
            nc.sync.dma_start(out=sqf_t[:], in_=sqf_s[e])
            labr_t = str_pool.tile([10, WINW], bf16, tag="labr")
            nc.sync.dma_start(out=labr_t[:], in_=labr_s[e])
            labl_t = str_pool.tile([10, WINW], bf16, tag="labl")
            nc.sync.dma_start(out=labl_t[:], in_=labl_s[e])
            bias_t = str_pool.tile([P, rt_n], fp32, tag="bias")
            nc.sync.dma_start(out=bias_t[:], in_=bias_s[e])
            for r in range(rt_n):
                blk = e * rt_n + r
                ms = slice(r * P, (r + 1) * P)
                psA = psA_pool.tile([P, WINW], fp32, tag="psA")
                psB = psB_pool.tile([P, WINW], fp32, tag="psB")
                if use_fp8:
                    for c in range(0, kch, 2):
                        nc.tensor.matmul(
                            psA[:], lhs_t[:, c : c + 2, ms], rhs_t[:, c : c + 2, :],
                            start=(c == 0), stop=False,
                            perf_mode=mybir.MatmulPerfMode.DoubleRow,
                        )
                else:
                    for c in range(kch):
                        nc.tensor.matmul(
                            psA[:], lhs_t[:, c, ms], rhs_t[:, c, :],
                            start=(c == 0), stop=False,
                        )
                nc.tensor.matmul(psA[:], ones2[:], sqf_t[:], start=False, stop=True)
                nc.tensor.matmul(psB[:], labl_t[:, ms], labr_t[:], start=True, stop=True)

                d2 = sc_pool.tile([P, WINW], fp32, tag="d2")
                nc.scalar.activation(
                    d2[:], psA[:], Act.Relu,
                    bias=bias_t[:, r : r + 1], scale=-1.0 / dfeat,
                )
                if e < 2:
                    # diagonal block: keep strictly-upper cells (col > row),
                    # zero the rest; host corrects the t=M per zeroed cell.
                    d2m = sc_pool.tile([P, WINW], fp32, tag="d2m")
                    nc.gpsimd.affine_select(
                        d2m[:], d2[:], pattern=[[1, WINW]],
                        compare_op=Alu.is_gt, fill=0.0,
                        base=-(r * P), channel_multiplier=-1,
                    )
                    d2 = d2m
                t_ = sc_pool.tile([P, WINW], fp32, tag="t")
                nc.scalar.activation(
                    t_[:], d2[:], Act.Relu,
                    bias=mbias[:], scale=-1.0,
                    accum_out=acct_t[:, blk : blk + 1],
                )
                u_ = sc_pool.tile([P, WINW], fp32, tag="u")
                nc.vector.scalar_tensor_tensor(
                    u_[:], in0=t_[:], scalar=-1.0, in1=d2[:],
                    op0=Alu.mult, op1=Alu.add,
                )
                y_ = sc_pool.tile([P, WINW], fp32, tag="y")
                nc.vector.scalar_tensor_tensor(
                    y_[:], in0=psB[:], scalar=-LAM, in1=u_[:],
                    op0=Alu.mult, op1=Alu.add,
                )
                w_ = sc_pool.tile([P, WINW], fp32, tag="w")
                nc.vector.tensor_scalar(
                    w_[:], y_[:], 0.0, None, Alu.max, Alu.add,
                    accum_out=accw_t[:, blk : blk + 1],
                )

        nc.sync.dma_start(out=acct_d[:], in_=acct_t[:])
        nc.sync.dma_start(out=accw_d[:], in_=accw_t[:])
    nc.compile()
    return nc


def host_prep_tri(predict, gt, b=B, dfeat=DFEAT, ncores=NCORES, use_fp8=False):
    """Per-core input maps for the triangle variant + host correction counts.

    use_fp8: quantize q to float8_e4m3 and debias the squared norms by the
    known quantization error energy, so E[d2_hat] == d2 (the raw fp8
    ||p^_i - p^_j||^2 overshoots by (||e_i||^2+||e_j||^2)/D otherwise)."""
    p = np.asarray(predict, np.float32).reshape(b, dfeat)
    q = p * np.float32(math.sqrt(2.0))
    qb = q.astype(ml_dtypes.float8_e4m3 if use_fp8 else BF16)
    qf = qb.astype(np.float32)
    if use_fp8:
        # true norms: cancels the row-common part of the quantization error
        # (E[q.e] != 0 for coarse RN grids); diagonal cells are masked anyway.
        stil = (0.5 * np.einsum("ij,ij->i", q.astype(np.float64), q.astype(np.float64))).astype(np.float32)
    else:
        stil = (0.5 * np.einsum("ij,ij->i", qf.astype(np.float64), qf.astype(np.float64))).astype(np.float32)
    smc = (stil - np.float32(C_OFF)).astype(np.float32)
    s_hi = smc.astype(BF16)
    s_lo = (smc - s_hi.astype(np.float32)).astype(BF16)
    sqf_full = np.stack([-s_hi, -s_lo]).astype(BF16)

    g = np.asarray(gt).reshape(-1).astype(np.int64)
    da, db_, dc = g % 10, (g // 10) % 10, g // 100
    onesb = np.ones(b, np.float32)
    labr_full = np.stack(
        [onesb, da, da * da, onesb, db_, db_ * db_, onesb, dc, dc * dc, onesb]
    ).astype(BF16)
    labl_full = np.stack(
        [da * da, -2.0 * da, onesb, db_ * db_, -2.0 * db_, onesb,
         dc * dc, -2.0 * dc, onesb, np.full(b, -2.0 / LAM, np.float32)]
    ).astype(BF16)
    bias_full = ((stil + np.float32(C_OFF)) / np.float32(dfeat)).astype(np.float32)
    qT = np.ascontiguousarray(qb.T)

    nblks = b // WINW
    del q, qf
    rt_n = WINW // P
    entries = tri_entries(ncores, nblks)
    in_maps = []
    for k in range(ncores):
        ents = entries[k]
        lhs_sx = np.stack([qT[:, rb * WINW:(rb + 1) * WINW] for rb, _ in ents])
        rhs_sx = np.stack([qT[:, cb * WINW:(cb + 1) * WINW] for _, cb in ents])
        sqf_sx = np.stack([sqf_full[:, cb * WINW:(cb + 1) * WINW] for _, cb in ents])
        labr_sx = np.stack([labr_full[:, cb * WINW:(cb + 1) * WINW] for _, cb in ents])
        labl_sx = np.stack([labl_full[:, rb * WINW:(rb + 1) * WINW] for rb, _ in ents])
        bias_sx = np.stack([
            np.ascontiguousarray(bias_full[rb * WINW:(rb + 1) * WINW].reshape(rt_n, P).T)
            for rb, _ in ents])
        in_maps.append({
            "lhs_s": np.ascontiguousarray(lhs_sx),
            "rhs_s": np.ascontiguousarray(rhs_sx),
            "sqf_s": np.ascontiguousarray(sqf_sx),
            "labr_s": np.ascontiguousarray(labr_sx),
            "labl_s": np.ascontiguousarray(labl_sx),
            "bias_s": np.ascontiguousarray(bias_sx),
        })
    n_label = int((np.bincount(g) ** 2).sum())
    n_masked = ncores * 2 * (WINW * (WINW + 1) // 2)   # j<=i cells zeroed per diag entry
    return in_maps, n_label, n_masked


def finish_tri(results, n_label, n_masked, b=B):
    s = 0.0
    for r in results:
        s += float(r["acc_t"].astype(np.float64).sum())
        s += float(r["acc_w"].astype(np.float64).sum())
    n_lab_strict = (n_label - b) // 2
    s_strict = s - MARGIN * n_masked - 2.0 * n_lab_strict
    loss = 2.0 * s_strict / (float(b) * (b - 1))
    return np.float32(loss)


def build_nc(b=B, dfeat=DFEAT, ncores=NCORES, nwin_override=None):
    import concourse.bass as bass
    import concourse.bacc as bacc
    import concourse.mybir as mybir
    from concourse.tile import TileContext

    fp32 = mybir.dt.float32
    bf16 = mybir.dt.bfloat16
    Act = mybir.ActivationFunctionType
    Alu = mybir.AluOpType

    rpc = b // ncores              # rows per core
    rt_n = rpc // P                # row tiles per core
    nwin = b // WINW               # column windows
    nwin_run = nwin if nwin_override is None else nwin_override
    kch = dfeat // P               # contraction chunks
    nblk = nwin * rt_n

    nc = bacc.Bacc()
    qT = nc.declare_dram_parameter("qT", [dfeat, b], bf16, isOutput=False)
    lhsT = nc.declare_dram_parameter("lhsT", [dfeat, rpc], bf16, isOutput=False)
    sqf = nc.declare_dram_parameter("sqf", [2, b], bf16, isOutput=False)
    labr = nc.declare_dram_parameter("labr", [10, b], bf16, isOutput=False)
    labl = nc.declare_dram_parameter("labl", [10, rpc], bf16, isOutput=False)
    bias = nc.declare_dram_parameter("bias", [P, rt_n], fp32, isOutput=False)
    acct_d = nc.declare_dram_parameter("acc_t", [P, nblk], fp32, isOutput=True)
    accw_d = nc.declare_dram_parameter("acc_w", [P, nblk], fp32, isOutput=True)

    with TileContext(nc) as tc, ExitStack() as ctx:
        const = ctx.enter_context(tc.tile_pool(name="const", bufs=1))
        lhs_pool = ctx.enter_context(tc.tile_pool(name="lhs", bufs=1))
        rhs_pool = ctx.enter_context(tc.tile_pool(name="rhs", bufs=3))
        sc_pool = ctx.enter_context(tc.tile_pool(name="scratch", bufs=3))
        psA_pool = ctx.enter_context(tc.tile_pool(name="psA", bufs=2, space="PSUM"))
        psB_pool = ctx.enter_context(tc.tile_pool(name="psB", bufs=2, space="PSUM"))
        acc_pool = ctx.enter_context(tc.tile_pool(name="acc", bufs=1))

        ones2 = const.tile([2, P], bf16)
        nc.any.memset(ones2[:], 1.0)
        mbias = const.tile([P, 1], fp32)
        nc.any.memset(mbias[:], MARGIN)
        bias_t = const.tile([P, rt_n], fp32)
        nc.sync.dma_start(out=bias_t[:], in_=bias[:])
        labl_t = const.tile([10, rpc], bf16)
        nc.sync.dma_start(out=labl_t[:], in_=labl[:])
        lhs_t = lhs_pool.tile([P, kch, rpc], bf16)
        nc.sync.dma_start(out=lhs_t[:], in_=lhsT.rearrange("(c p) m -> p c m", p=P))

        acct_t = acc_pool.tile([P, nblk], fp32)
        accw_t = acc_pool.tile([P, nblk], fp32)

        qT_r = qT.rearrange("(c p) n -> p c n", p=P)

        for w in range(nwin_run):
            cs = slice(w * WINW, (w + 1) * WINW)
            rhs_t = rhs_pool.tile([P, kch, WINW], bf16, tag="rhs")
            nc.sync.dma_start(out=rhs_t[:], in_=qT_r[:, :, cs])
            sqf_t = rhs_pool.tile([2, WINW], bf16, tag="sqf")
            nc.sync.dma_start(out=sqf_t[:], in_=sqf[:, cs])
            labr_t = rhs_pool.tile([10, WINW], bf16, tag="labr")
            nc.sync.dma_start(out=labr_t[:], in_=labr[:, cs])
            for r in range(rt_n):
                blk = w * rt_n + r
                ms = slice(r * P, (r + 1) * P)
                psA = psA_pool.tile([P, WINW], fp32, tag="psA")
                psB = psB_pool.tile([P, WINW], fp32, tag="psB")
                for c in range(kch):
                    nc.tensor.matmul(
                        psA[:], lhs_t[:, c, ms], rhs_t[:, c, :],
                        start=(c == 0), stop=False,
                    )
                nc.tensor.matmul(psA[:], ones2[:], sqf_t[:], start=False, stop=True)
                nc.tensor.matmul(psB[:], labl_t[:, ms], labr_t[:], start=True, stop=True)

                d2 = sc_pool.tile([P, WINW], fp32, tag="d2")
                nc.scalar.activation(
                    d2[:], psA[:], Act.Relu,
                    bias=bias_t[:, r : r + 1], scale=-1.0 / dfeat,
                )
                t_ = sc_pool.tile([P, WINW], fp32, tag="t")
                nc.scalar.activation(
                    t_[:], d2[:], Act.Relu,
                    bias=mbias[:], scale=-1.0,
                    accum_out=acct_t[:, blk : blk + 1],
                )
                u_ = sc_pool.tile([P, WINW], fp32, tag="u")
                nc.vector.scalar_tensor_tensor(
                    u_[:], in0=t_[:], scalar=-1.0, in1=d2[:],
                    op0=Alu.mult, op1=Alu.add,
                )
                y_ = sc_pool.tile([P, WINW], fp32, tag="y")
                nc.vector.scalar_tensor_tensor(
                    y_[:], in0=psB[:], scalar=-LAM, in1=u_[:],
                    op0=Alu.mult, op1=Alu.add,
                )
                w_ = sc_pool.tile([P, WINW], fp32, tag="w")
                nc.vector.tensor_scalar(
                    w_[:], y_[:], 0.0, None, Alu.max, Alu.add,
                    accum_out=accw_t[:, blk : blk + 1],
                )

        nc.sync.dma_start(out=acct_d[:], in_=acct_t[:])
        nc.sync.dma_start(out=accw_d[:], in_=accw_t[:])
    nc.compile()
    return nc


def host_prep(predict, gt, b=B, dfeat=DFEAT, ncores=NCORES):
    """Build per-core input maps + the host-side label-pair count."""
    p = np.asarray(predict, np.float32).reshape(b, dfeat)
    q = p * np.float32(math.sqrt(2.0))
    qb = q.astype(BF16)
    qf = qb.astype(np.float32)
    stil = (0.5 * np.einsum("ij,ij->i", qf.astype(np.float64), qf.astype(np.float64))).astype(np.float32)
    smc = (stil - np.float32(C_OFF)).astype(np.float32)
    s_hi = smc.astype(BF16)
    s_lo = (smc - s_hi.astype(np.float32)).astype(BF16)
    sqf_full = np.stack([-s_hi, -s_lo]).astype(BF16)          # [2, B]

    g = np.asarray(gt).reshape(-1).astype(np.int64)
    da, db_, dc = g % 10, (g // 10) % 10, g // 100
    onesb = np.ones(b, np.float32)
    labr_full = np.stack(
        [onesb, da, da * da, onesb, db_, db_ * db_, onesb, dc, dc * dc, onesb]
    ).astype(BF16)                                            # [10, B]
    labl_full = np.stack(
        [da * da, -2.0 * da, onesb, db_ * db_, -2.0 * db_, onesb,
         dc * dc, -2.0 * dc, onesb, np.full(b, -2.0 / LAM, np.float32)]
    ).astype(BF16)                                            # [10, B]
    bias_full = ((stil + np.float32(C_OFF)) / np.float32(dfeat)).astype(np.float32)

    qT = np.ascontiguousarray(qb.T)                           # [D, B] bf16
    rpc = b // ncores
    rt_n = rpc // P
    in_maps = []
    for k in range(ncores):
        rs = slice(k * rpc, (k + 1) * rpc)
        in_maps.append({
            "qT": qT,
            "lhsT": np.ascontiguousarray(qT[:, rs]),
            "sqf": sqf_full,
            "labr": labr_full,
            "labl": np.ascontiguousarray(labl_full[:, rs]),
            "bias": np.ascontiguousarray(bias_full[rs].reshape(rt_n, P).T),
        })
    n_label = int((np.bincount(g) ** 2).sum())
    return in_maps, n_label


def finish(results, n_label, b=B):
    s = 0.0
    for r in results:
        s += float(r["acc_t"].astype(np.float64).sum())
        s += float(r["acc_w"].astype(np.float64).sum())
    loss = (s - 2.0 * n_label) / (float(b) * (b - 1))
    return np.float32(loss)


def kernel(predict, gt):
    global LAST_EXEC_NS, LAST_RESULTS
    from concourse.bass_utils import run_bass_kernel_spmd

    nc = build_tri_nc()
    in_maps, n_label, n_masked = host_prep_tri(predict, gt)
    res = run_bass_kernel_spmd(nc, in_maps, list(range(NCORES)))
    LAST_EXEC_NS = res.exec_time_ns
    LAST_RESULTS = res
    return finish_tri(res.results, n_label, n_masked)



# revision 9
# speedup vs baseline: 3.0685x; 3.0685x over previous
"""ContrastiveLoss Trainium2 kernel (8 NeuronCores, SPMD).

Math: reference computes, over p = predict.reshape(B, D):
    d2_ij  = max(||p_i||^2 + ||p_j||^2 - 2 p_i.p_j, 0) / D
    lab_ij = [g_i == g_j]
    term   = lab*d2 + (1-lab)*relu(M - d2),  M = 2
    loss   = 2 * sum_{j>=i} term / (B*(B-1))

Device formulation (full matrix, diagonal self-cancelling):
  q = bf16(sqrt(2)*p); stil_i = (q_i.q_i)/2  (host fp32)
  PSUM_A = q_i.q_j - (stil_j - C)            (bf16 GEMM K=2048 + K=2 feature GEMM)
  d2     = relu(-PSUM_A/D + (stil_i + C)/D)  == ||p^_i - p^_j||^2/D  >= 0 exactly
  t      = relu(M - d2)                      (scalar engine, accum_out -> col sums)
  PSUM_B = dg2 - 2/LAM  where dg2 = sum_digit (d_i - d_j)^2  (exact bf16 GEMM K=10)
  w      = relu((d2 - t) + 2 - LAM*dg2)      (vector engine, accum_out -> col sums)
  sum over full matrix:  S = sum t + sum w  satisfies
  loss   = (S - 2*N_label) / (B*(B-1)),  N_label = sum_c count_c^2 (ordered, incl diag)
Each core owns 1024 rows x all 8192 cols; host sums 8x2 [128,128] accumulators.
"""

import os
import math
from contextlib import ExitStack

import numpy as np
import ml_dtypes

BF16 = ml_dtypes.bfloat16

B = 8192
DFEAT = 2048
NCORES = 8
P = 128
WINW = 512
MARGIN = 2.0
C_OFF = 2048.0
LAM = 64.0

LAST_EXEC_NS = None
LAST_RESULTS = None


def tri_entries(ncores=NCORES, nblks=16):
    """Per-core list of (row_block, col_block) in 512-units: 2 diagonal entries
    first, then 15 plain upper-triangle entries. Uniform length for every core."""
    per_core = []
    for k in range(ncores):
        a, bb = k, nblks - 1 - k
        ents = [(a, a), (bb, bb)]
        ents += [(a, c) for c in range(a + 1, nblks)]
        ents += [(bb, c) for c in range(bb + 1, nblks)]
        assert len(ents) == nblks + 1
        per_core.append(ents)
    return per_core


def build_tri_nc(b=B, dfeat=DFEAT, ncores=NCORES, use_fp8=False, reps=1):
    """Triangle variant: 17 host-packed (lhs, rhs) stream entries per core,
    4 row-tiles each; entries 0,1 are diagonal blocks masked to j > i.
    use_fp8: main GEMM in float8e4 with DoubleRow (2 K-chunks per matmul).
    reps: repeat the whole body (timing builds; outputs are overwritten
    identically each rep)."""
    import concourse.bacc as bacc
    import concourse.mybir as mybir
    from concourse.tile import TileContext

    fp32 = mybir.dt.float32
    bf16 = mybir.dt.bfloat16
    qdt = mybir.dt.float8e4 if use_fp8 else bf16
    Act = mybir.ActivationFunctionType
    Alu = mybir.AluOpType

    kch = dfeat // P               # contraction chunks
    nent = (b // WINW) + 1         # 17 stream entries
    rt_n = WINW // P               # 4 row tiles per entry
    nblk = nent * rt_n             # 68 accumulator columns

    nc = bacc.Bacc()
    lhs_s = nc.declare_dram_parameter("lhs_s", [nent, dfeat, WINW], qdt, isOutput=False)
    rhs_s = nc.declare_dram_parameter("rhs_s", [nent, dfeat, WINW], qdt, isOutput=False)
    sqf_s = nc.declare_dram_parameter("sqf_s", [nent, 2, WINW], bf16, isOutput=False)
    labr_s = nc.declare_dram_parameter("labr_s", [nent, 10, WINW], bf16, isOutput=False)
    labl_s = nc.declare_dram_parameter("labl_s", [nent, 10, WINW], bf16, isOutput=False)
    bias_s = nc.declare_dram_parameter("bias_s", [nent, P, rt_n], fp32, isOutput=False)
    acct_d = nc.declare_dram_parameter("acc_t", [P, nblk], fp32, isOutput=True)
    accw_d = nc.declare_dram_parameter("acc_w", [P, nblk], fp32, isOutput=True)

    with TileContext(nc) as tc, ExitStack() as ctx:
        const = ctx.enter_context(tc.tile_pool(name="const", bufs=1))
        str_pool = ctx.enter_context(tc.tile_pool(name="streams", bufs=3))
        sc_pool = ctx.enter_context(tc.tile_pool(name="scratch", bufs=3))
        psA_pool = ctx.enter_context(tc.tile_pool(name="psA", bufs=2, space="PSUM"))
        psB_pool = ctx.enter_context(tc.tile_pool(name="psB", bufs=2, space="PSUM"))
        acc_pool = ctx.enter_context(tc.tile_pool(name="acc", bufs=1))

        ones2 = const.tile([2, P], bf16)
        nc.any.memset(ones2[:], 1.0)
        mbias = const.tile([P, 1], fp32)
        nc.any.memset(mbias[:], MARGIN)
        acct_t = acc_pool.tile([P, nblk], fp32)
        accw_t = acc_pool.tile([P, nblk], fp32)

        lhs_r = lhs_s.rearrange("e (c p) n -> e p c n", p=P)
        rhs_r = rhs_s.rearrange("e (c p) n -> e p c n", p=P)

        for e in [e for _ in range(reps) for e in range(nent)]:
            lhs_t = str_pool.tile([P, kch, WINW], qdt, tag="lhs")
            nc.sync.dma_start(out=lhs_t[:], in_=lhs_r[e])
            rhs_t = str_pool.tile([P, kch, WINW], qdt, tag="rhs")
            nc.sync.dma_start(out=rhs_t[:], in_=rhs_r[e])
            sqf_t = str_pool.tile([2, WINW], bf16, tag="sqf")
            nc.sync.dma_start(out=sqf_t[:], in_=sqf_s[e])
            labr_t = str_pool.tile([10, WINW], bf16, tag="labr")
            nc.sync.dma_start(out=labr_t[:], in_=labr_s[e])
            labl_t = str_pool.tile([10, WINW], bf16, tag="labl")
            nc.sync.dma_start(out=labl_t[:], in_=labl_s[e])
            bias_t = str_pool.tile([P, rt_n], fp32, tag="bias")
            nc.sync.dma_start(out=bias_t[:], in_=bias_s[e])
            for r in range(rt_n):
                blk = e * rt_n + r
                ms = slice(r * P, (r + 1) * P)
                psA = psA_pool.tile([P, WINW], fp32, tag="psA")
                psB = psB_pool.tile([P, WINW], fp32, tag="psB")
                if use_fp8:
                    for c in range(0, kch, 2):
                        nc.tensor.matmul(
                            psA[:], lhs_t[:, c : c + 2, ms], rhs_t[:, c : c + 2, :],
                            start=(c == 0), stop=False,
                            perf_mode=mybir.MatmulPerfMode.DoubleRow,
                        )
                else:
                    for c in range(kch):
                        nc.tensor.matmul(
                            psA[:], lhs_t[:, c, ms], rhs_t[:, c, :],
                            start=(c == 0), stop=False,
                        )
                nc.tensor.matmul(psA[:], ones2[:], sqf_t[:], start=False, stop=True)
                nc.tensor.matmul(psB[:], labl_t[:, ms], labr_t[:], start=True, stop=True)

                d2 = sc_pool.tile([P, WINW], fp32, tag="d2")
                nc.scalar.activation(
                    d2[:], psA[:], Act.Relu,
                    bias=bias_t[:, r : r + 1], scale=-1.0 / dfeat,
                )
                if e < 2:
                    # diagonal block: keep strictly-upper cells (col > row),
                    # zero the rest; host corrects the t=M per zeroed cell.
                    d2m = sc_pool.tile([P, WINW], fp32, tag="d2m")
                    nc.gpsimd.affine_select(
                        d2m[:], d2[:], pattern=[[1, WINW]],
                        compare_op=Alu.is_gt, fill=0.0,
                        base=-(r * P), channel_multiplier=-1,
                    )
                    d2 = d2m
                t_ = sc_pool.tile([P, WINW], fp32, tag="t")
                nc.scalar.activation(
                    t_[:], d2[:], Act.Relu,
                    bias=mbias[:], scale=-1.0,
                    accum_out=acct_t[:, blk : blk + 1],
                )
                u_ = sc_pool.tile([P, WINW], fp32, tag="u")
                nc.vector.scalar_tensor_tensor(
                    u_[:], in0=t_[:], scalar=-1.0, in1=d2[:],
                    op0=Alu.mult, op1=Alu.add,
                )
                y_ = sc_pool.tile([P, WINW], fp32, tag="y")
                nc.vector.scalar_tensor_tensor(
                    y_[:], in0=psB[:], scalar=-LAM, in1=u_[:],
                    op0=Alu.mult, op1=Alu.add,
                )
                w_ = sc_pool.tile([P, WINW], fp32, tag="w")
                nc.vector.tensor_scalar(
                    w_[:], y_[:], 0.0, None, Alu.max, Alu.add,
                    accum_out=accw_t[:, blk : blk + 1],
                )

        nc.sync.dma_start(out=acct_d[:], in_=acct_t[:])
        nc.sync.dma_start(out=accw_d[:], in_=accw_t[:])
    nc.compile()
    return nc


def host_prep_tri(predict, gt, b=B, dfeat=DFEAT, ncores=NCORES, use_fp8=False):
    """Per-core input maps for the triangle variant + host correction counts.

    use_fp8: quantize q to float8_e4m3 and debias the squared norms by the
    known quantization error energy, so E[d2_hat] == d2 (the raw fp8
    ||p^_i - p^_j||^2 overshoots by (||e_i||^2+||e_j||^2)/D otherwise)."""
    p = np.asarray(predict, np.float32).reshape(b, dfeat)
    q = p * np.float32(math.sqrt(2.0))
    qb = q.astype(ml_dtypes.float8_e4m3 if use_fp8 else BF16)
    qf = qb.astype(np.float32)
    if use_fp8:
        # true norms: cancels the row-common part of the quantization error
        # (E[q.e] != 0 for coarse RN grids); diagonal cells are masked anyway.
        stil = (0.5 * np.einsum("ij,ij->i", q.astype(np.float64), q.astype(np.float64))).astype(np.float32)
    else:
        stil = (0.5 * np.einsum("ij,ij->i", qf.astype(np.float64), qf.astype(np.float64))).astype(np.float32)
    smc = (stil - np.float32(C_OFF)).astype(np.float32)
    s_hi = smc.astype(BF16)
    s_lo = (smc - s_hi.astype(np.float32)).astype(BF16)
    sqf_full = np.stack([-s_hi, -s_lo]).astype(BF16)

    g = np.asarray(gt).reshape(-1).astype(np.int64)
    da, db_, dc = g % 10, (g // 10) % 10, g // 100
    onesb = np.ones(b, np.float32)
    labr_full = np.stack(
        [onesb, da, da * da, onesb, db_, db_ * db_, onesb, dc, dc * dc, onesb]
    ).astype(BF16)
    labl_full = np.stack(
        [da * da, -2.0 * da, onesb, db_ * db_, -2.0 * db_, onesb,
         dc * dc, -2.0 * dc, onesb, np.full(b, -2.0 / LAM, np.float32)]
    ).astype(BF16)
    bias_full = ((stil + np.float32(C_OFF)) / np.float32(dfeat)).astype(np.float32)
    qT = np.ascontiguousarray(qb.T)

    nblks = b // WINW
    del q, qf
    rt_n = WINW // P
    entries = tri_entries(ncores, nblks)
    in_maps = []
    for k in range(ncores):
        ents = entries[k]
        lhs_sx = np.stack([qT[:, rb * WINW:(rb + 1) * WINW] for rb, _ in ents])
        rhs_sx = np.stack([qT[:, cb * WINW:(cb + 1) * WINW] for _, cb in ents])
        sqf_sx = np.stack([sqf_full[:, cb * WINW:(cb + 1) * WINW] for _, cb in ents])
        labr_sx = np.stack([labr_full[:, cb * WINW:(cb + 1) * WINW] for _, cb in ents])
        labl_sx = np.stack([labl_full[:, rb * WINW:(rb + 1) * WINW] for rb, _ in ents])
        bias_sx = np.stack([
            np.ascontiguousarray(bias_full[rb * WINW:(rb + 1) * WINW].reshape(rt_n, P).T)
            for rb, _ in ents])
        in_maps.append({
            "lhs_s": np.ascontiguousarray(lhs_sx),
            "rhs_s": np.ascontiguousarray(rhs_sx),
            "sqf_s": np.ascontiguousarray(sqf_sx),
            "labr_s": np.ascontiguousarray(labr_sx),
            "labl_s": np.ascontiguousarray(labl_sx),
            "bias_s": np.ascontiguousarray(bias_sx),
        })
    n_label = int((np.bincount(g) ** 2).sum())
    n_masked = ncores * 2 * (WINW * (WINW + 1) // 2)   # j<=i cells zeroed per diag entry
    return in_maps, n_label, n_masked


def finish_tri(results, n_label, n_masked, b=B):
    s = 0.0
    for r in results:
        s += float(r["acc_t"].astype(np.float64).sum())
        s += float(r["acc_w"].astype(np.float64).sum())
    n_lab_strict = (n_label - b) // 2
    s_strict = s - MARGIN * n_masked - 2.0 * n_lab_strict
    loss = 2.0 * s_strict / (float(b) * (b - 1))
    return np.float32(loss)


def cyc_entries(k, nblks=16):
    """Cyclic-uniform block assignment for core k: row blocks (k, k+8).
    A-entries d=0..8 (col (k+d)%16), B-entries d=0..7 (col (k+8+d)%16).
    Every unordered block pair {r,c} is covered exactly once; entries 0 and
    9 are the diagonal blocks. Identical program shape for every core."""
    a, bb = k, k + nblks // 2
    ents = [(a, (a + d) % nblks) for d in range(nblks // 2 + 1)]
    ents += [(bb, (bb + d) % nblks) for d in range(nblks // 2)]
    return ents


def dedup_ldweights(nc):
    """Remove InstLdweights whose weights AP repeats the immediately
    preceding InstLdweights (same tensor/offset/pattern/perf_mode) with no
    other PE weight load in between. The PE weight registers persist across
    matmuls, so the repeated load is redundant. Only drops instructions with
    no sync updates and whose waits are a subset of the kept LDW's waits
    (identical tile => identical waits in practice; else keep)."""
    import concourse.mybir as mybir

    n_drop = 0
    for blk in nc.m.functions[0].blocks:
        insts = blk.instructions
        prev_key = None
        prev_wait_names = None
        keep = []
        for inst in insts:
            if isinstance(inst, mybir.InstLdweights):
                ap = inst.ins[0]
                key = (repr(ap), repr(inst.perf_mode))
                si = inst.sync_info
                waits = tuple(sorted(repr(w) for w in si.on_wait)) if si else ()
                upds = tuple(si.on_update) if si else ()
                if (key == prev_key and not upds
                        and set(waits) <= set(prev_wait_names or ())):
                    n_drop += 1
                    continue
                prev_key = key
                prev_wait_names = waits
            elif isinstance(inst, mybir.InstMatmult):
                pass  # matmuls don't disturb loaded weights
            elif inst.engine == mybir.EngineType.PE:
                prev_key = None
            keep.append(inst)
        if n_drop:
            insts[:] = keep
    return n_drop


def build_cyc_nc(b=B, dfeat=DFEAT, ncores=NCORES, use_fp8=True, group=1,
                 dedup=False, reps=1):
    """Cyclic-uniform variant: 2 resident lhs row-blocks per core, 17
    streamed rhs windows, contiguous per-partition DMA lines. Entries 0 and
    9 are diagonal (masked); the rest use a fused 2-scalar+2-vector
    elementwise pipeline with no d2 clamp (off-diagonal d2 ~ 2, never near
    0, so the clamp only ever mattered on masked diagonal cells).

    group>1: process `group` entries sharing one lhs block together so the
    c-loop can reuse the stationary weights across `group` matmuls; with
    dedup=True the redundant InstLdweights are stripped post-build."""
    import concourse.bacc as bacc
    import concourse.mybir as mybir
    from concourse.tile import TileContext

    fp32 = mybir.dt.float32
    bf16 = mybir.dt.bfloat16
    qdt = mybir.dt.float8e4 if use_fp8 else bf16
    Act = mybir.ActivationFunctionType
    Alu = mybir.AluOpType

    kch = dfeat // P               # 16 contraction chunks
    nent = (b // WINW) + 1         # 17 entries
    rt_n = WINW // P               # 4 row tiles
    nblk = nent * rt_n             # 68 accumulator columns

    nc = bacc.Bacc()
    lhs2 = nc.declare_dram_parameter("lhs2", [2, P, kch * WINW], qdt, isOutput=False)
    rhs_s = nc.declare_dram_parameter("rhs_s", [nent, P, kch * WINW], qdt, isOutput=False)
    sqf_s = nc.declare_dram_parameter("sqf_s", [nent, 2, WINW], bf16, isOutput=False)
    labr_s = nc.declare_dram_parameter("labr_s", [nent, 10, WINW], bf16, isOutput=False)
    labl2 = nc.declare_dram_parameter("labl2", [2, 10, WINW], bf16, isOutput=False)
    sbias2 = nc.declare_dram_parameter("sbias2", [2, P, rt_n], fp32, isOutput=False)
    tbias2 = nc.declare_dram_parameter("tbias2", [2, P, rt_n], fp32, isOutput=False)
    acct_d = nc.declare_dram_parameter("acc_t", [P, nblk], fp32, isOutput=True)
    accw_d = nc.declare_dram_parameter("acc_w", [P, nblk], fp32, isOutput=True)

    # entry -> lhs block (0=A rows k, 1=B rows k+8); diagonal entries: 0, 9
    ent_blk = [0] * 9 + [1] * 8
    diag_ents = (0, 9)

    with TileContext(nc) as tc, ExitStack() as ctx:
        const = ctx.enter_context(tc.tile_pool(name="const", bufs=1))
        str_pool = ctx.enter_context(tc.tile_pool(name="streams", bufs=max(3, group + 1)))
        sc_pool = ctx.enter_context(tc.tile_pool(name="scratch", bufs=3))
        psA_pool = ctx.enter_context(tc.tile_pool(name="psA", bufs=2, space="PSUM"))
        psB_pool = ctx.enter_context(tc.tile_pool(name="psB", bufs=2, space="PSUM"))
        acc_pool = ctx.enter_context(tc.tile_pool(name="acc", bufs=1))

        ones2 = const.tile([2, P], bf16)
        nc.any.memset(ones2[:], 1.0)
        mbias = const.tile([P, 1], fp32)
        nc.any.memset(mbias[:], MARGIN)
        lhs_r2 = lhs2.rearrange("b p (c n) -> b p c n", n=WINW)
        lhs_t, labl_t, sbias_t, tbias_t = [], [], [], []
        for bi in range(2):
            lt = const.tile([P, kch, WINW], qdt, name=f"lhs{bi}")
            nc.sync.dma_start(out=lt[:], in_=lhs_r2[bi])
            lhs_t.append(lt)
            ll = const.tile([10, WINW], bf16, name=f"labl{bi}")
            nc.sync.dma_start(out=ll[:], in_=labl2[bi])
            labl_t.append(ll)
            sb = const.tile([P, rt_n], fp32, name=f"sbias{bi}")
            nc.sync.dma_start(out=sb[:], in_=sbias2[bi])
            sbias_t.append(sb)
            tb = const.tile([P, rt_n], fp32, name=f"tbias{bi}")
            nc.sync.dma_start(out=tb[:], in_=tbias2[bi])
            tbias_t.append(tb)

        acct_t = acc_pool.tile([P, nblk], fp32)
        accw_t = acc_pool.tile([P, nblk], fp32)

        rhs_r = rhs_s.rearrange("e p (c n) -> e p c n", n=WINW)

        def do_entry_tail(e, r, psA, psB):
            """Elementwise pipeline for row-tile r of entry e (PSUMs ready)."""
            blk = e * rt_n + r
            bi = ent_blk[e]
            if e in diag_ents:
                d2 = sc_pool.tile([P, WINW], fp32, tag="d2")
                nc.scalar.activation(
                    d2[:], psA[:], Act.Relu,
                    bias=sbias_t[bi][:, r : r + 1], scale=-1.0 / dfeat,
                )
                d2m = sc_pool.tile([P, WINW], fp32, tag="d2m")
                nc.gpsimd.affine_select(
                    d2m[:], d2[:], pattern=[[1, WINW]],
                    compare_op=Alu.is_gt, fill=0.0,
                    base=-(r * P), channel_multiplier=-1,
                )
                t_ = sc_pool.tile([P, WINW], fp32, tag="t")
                nc.scalar.activation(
                    t_[:], d2m[:], Act.Relu,
                    bias=mbias[:], scale=-1.0,
                    accum_out=acct_t[:, blk : blk + 1],
                )
                u_ = sc_pool.tile([P, WINW], fp32, tag="u")
                nc.vector.scalar_tensor_tensor(
                    u_[:], in0=t_[:], scalar=-1.0, in1=d2m[:],
                    op0=Alu.mult, op1=Alu.add,
                )
                y_ = sc_pool.tile([P, WINW], fp32, tag="y")
                nc.vector.scalar_tensor_tensor(
                    y_[:], in0=psB[:], scalar=-LAM, in1=u_[:],
                    op0=Alu.mult, op1=Alu.add,
                )
                w_ = sc_pool.tile([P, WINW], fp32, tag="w")
                nc.vector.tensor_scalar(
                    w_[:], y_[:], 0.0, None, Alu.max, Alu.add,
                    accum_out=accw_t[:, blk : blk + 1],
                )
            else:
                # fused: t = relu(psA/D + tb); v = -psA/D - t;
                # y = -LAM*psB + v; w = relu(y + sb)
                t_ = sc_pool.tile([P, WINW], fp32, tag="t")
                nc.scalar.activation(
                    t_[:], psA[:], Act.Relu,
                    bias=tbias_t[bi][:, r : r + 1], scale=1.0 / dfeat,
                    accum_out=acct_t[:, blk : blk + 1],
                )
                v_ = sc_pool.tile([P, WINW], fp32, tag="u")
                nc.vector.scalar_tensor_tensor(
                    v_[:], in0=psA[:], scalar=-1.0 / dfeat, in1=t_[:],
                    op0=Alu.mult, op1=Alu.subtract,
                )
                y_ = sc_pool.tile([P, WINW], fp32, tag="y")
                nc.vector.scalar_tensor_tensor(
                    y_[:], in0=psB[:], scalar=-LAM, in1=v_[:],
                    op0=Alu.mult, op1=Alu.add,
                )
                w_ = sc_pool.tile([P, WINW], fp32, tag="w")
                nc.scalar.activation(
                    w_[:], y_[:], Act.Relu,
                    bias=sbias_t[bi][:, r : r + 1], scale=1.0,
                    accum_out=accw_t[:, blk : blk + 1],
                )

        def load_streams(e):
            rhs_t = str_pool.tile([P, kch, WINW], qdt, tag="rhs")
            nc.sync.dma_start(out=rhs_t[:], in_=rhs_r[e])
            sqf_t = str_pool.tile([2, WINW], bf16, tag="sqf")
            nc.sync.dma_start(out=sqf_t[:], in_=sqf_s[e])
            labr_t = str_pool.tile([10, WINW], bf16, tag="labr")
            nc.sync.dma_start(out=labr_t[:], in_=labr_s[e])
            return rhs_t, sqf_t, labr_t

        def small_mms(e, r, psA, psB, sqf_t, labr_t):
            bi = ent_blk[e]
            ms = slice(r * P, (r + 1) * P)
            nc.tensor.matmul(psA[:], ones2[:], sqf_t[:], start=False, stop=True)
            nc.tensor.matmul(psB[:], labl_t[bi][:, ms], labr_t[:], start=True, stop=True)

        # group entries by shared lhs block: A entries 0..8, B entries 9..16
        groups = []
        for base, n in ((0, 9), (9, 8)):
            ents = list(range(base, base + n))
            for i in range(0, n, group):
                groups.append(ents[i:i + group])

        for _ in range(reps):
            for g in groups:
                streams = {e: load_streams(e) for e in g}
                bi = ent_blk[g[0]]
                for r in range(rt_n):
                    ms = slice(r * P, (r + 1) * P)
                    psAs = {e: psA_pool.tile([P, WINW], fp32, tag=f"psA{i}",
                                             name=f"psA{i}")
                            for i, e in enumerate(g)}
                    if use_fp8:
                        for c in range(0, kch, 2):
                            for e in g:
                                nc.tensor.matmul(
                                    psAs[e][:], lhs_t[bi][:, c : c + 2, ms],
                                    streams[e][0][:, c : c + 2, :],
                                    start=(c == 0), stop=False,
                                    perf_mode=mybir.MatmulPerfMode.DoubleRow,
                                )
                    else:
                        for c in range(kch):
                            for e in g:
                                nc.tensor.matmul(
                                    psAs[e][:], lhs_t[bi][:, c, ms],
                                    streams[e][0][:, c, :],
                                    start=(c == 0), stop=False,
                                )
                    for e in g:
                        psB = psB_pool.tile([P, WINW], fp32, tag="psB")
                        small_mms(e, r, psAs[e], psB, streams[e][1], streams[e][2])
                        do_entry_tail(e, r, psAs[e], psB)

        nc.sync.dma_start(out=acct_d[:], in_=acct_t[:])
        nc.sync.dma_start(out=accw_d[:], in_=accw_t[:])
    if dedup:
        n = dedup_ldweights(nc)
        assert n > 0
    nc.compile()
    return nc


def host_prep_cyc(predict, gt, b=B, dfeat=DFEAT, ncores=NCORES, use_fp8=True):
    """Per-core input maps for the cyclic-uniform variant."""
    p = np.asarray(predict, np.float32).reshape(b, dfeat)
    q = p * np.float32(math.sqrt(2.0))
    qb = q.astype(ml_dtypes.float8_e4m3 if use_fp8 else BF16)
    if use_fp8:
        stil = (0.5 * np.einsum("ij,ij->i", q.astype(np.float64), q.astype(np.float64))).astype(np.float32)
    else:
        qf = qb.astype(np.float32)
        stil = (0.5 * np.einsum("ij,ij->i", qf.astype(np.float64), qf.astype(np.float64))).astype(np.float32)
    smc = (stil - np.float32(C_OFF)).astype(np.float32)
    s_hi = smc.astype(BF16)
    s_lo = (smc - s_hi.astype(np.float32)).astype(BF16)
    sqf_full = np.stack([-s_hi, -s_lo]).astype(BF16)

    g = np.asarray(gt).reshape(-1).astype(np.int64)
    da, db_, dc = g % 10, (g // 10) % 10, g // 100
    onesb = np.ones(b, np.float32)
    labr_full = np.stack(
        [onesb, da, da * da, onesb, db_, db_ * db_, onesb, dc, dc * dc, onesb]
    ).astype(BF16)
    labl_full = np.stack(
        [da * da, -2.0 * da, onesb, db_ * db_, -2.0 * db_, onesb,
         dc * dc, -2.0 * dc, onesb, np.full(b, -2.0 / LAM, np.float32)]
    ).astype(BF16)
    sbias_full = ((stil + np.float32(C_OFF)) / np.float32(dfeat)).astype(np.float32)
    tbias_full = (np.float32(MARGIN) - sbias_full).astype(np.float32)

    kch = dfeat // P
    rt_n = WINW // P
    # contiguous pack: [P, kch*WINW] per 512-row block, lines contiguous
    qT = np.ascontiguousarray(qb.T)                     # [D, B]
    nblks = b // WINW

    def pack_block(cb):
        blk = qT[:, cb * WINW:(cb + 1) * WINW]          # [D, W]
        return np.ascontiguousarray(
            blk.reshape(kch, P, WINW).transpose(1, 0, 2).reshape(P, kch * WINW))

    packed = [pack_block(cb) for cb in range(nblks)]

    in_maps = []
    for k in range(ncores):
        ents = cyc_entries(k, nblks)
        rA, rB = ents[0][0], ents[9][0]
        cols = [c for _, c in ents]
        lhs2 = np.stack([packed[rA], packed[rB]])
        rhs_sx = np.stack([packed[c] for c in cols])
        sqf_sx = np.stack([sqf_full[:, c * WINW:(c + 1) * WINW] for c in cols])
        labr_sx = np.stack([labr_full[:, c * WINW:(c + 1) * WINW] for c in cols])
        labl2_x = np.stack([labl_full[:, r * WINW:(r + 1) * WINW] for r in (rA, rB)])
        sb2 = np.stack([
            np.ascontiguousarray(sbias_full[r * WINW:(r + 1) * WINW].reshape(rt_n, P).T)
            for r in (rA, rB)])
        tb2 = np.stack([
            np.ascontiguousarray(tbias_full[r * WINW:(r + 1) * WINW].reshape(rt_n, P).T)
            for r in (rA, rB)])
        in_maps.append({
            "lhs2": np.ascontiguousarray(lhs2),
            "rhs_s": np.ascontiguousarray(rhs_sx),
            "sqf_s": np.ascontiguousarray(sqf_sx),
            "labr_s": np.ascontiguousarray(labr_sx),
            "labl2": np.ascontiguousarray(labl2_x),
            "sbias2": sb2,
            "tbias2": tb2,
        })
    n_label = int((np.bincount(g) ** 2).sum())
    n_masked = ncores * 2 * (WINW * (WINW + 1) // 2)
    return in_maps, n_label, n_masked


def build_nc(b=B, dfeat=DFEAT, ncores=NCORES, nwin_override=None):
    import concourse.bass as bass
    import concourse.bacc as bacc
    import concourse.mybir as mybir
    from concourse.tile import TileContext

    fp32 = mybir.dt.float32
    bf16 = mybir.dt.bfloat16
    Act = mybir.ActivationFunctionType
    Alu = mybir.AluOpType

    rpc = b // ncores              # rows per core
    rt_n = rpc // P                # row tiles per core
    nwin = b // WINW               # column windows
    nwin_run = nwin if nwin_override is None else nwin_override
    kch = dfeat // P               # contraction chunks
    nblk = nwin * rt_n

    nc = bacc.Bacc()
    qT = nc.declare_dram_parameter("qT", [dfeat, b], bf16, isOutput=False)
    lhsT = nc.declare_dram_parameter("lhsT", [dfeat, rpc], bf16, isOutput=False)
    sqf = nc.declare_dram_parameter("sqf", [2, b], bf16, isOutput=False)
    labr = nc.declare_dram_parameter("labr", [10, b], bf16, isOutput=False)
    labl = nc.declare_dram_parameter("labl", [10, rpc], bf16, isOutput=False)
    bias = nc.declare_dram_parameter("bias", [P, rt_n], fp32, isOutput=False)
    acct_d = nc.declare_dram_parameter("acc_t", [P, nblk], fp32, isOutput=True)
    accw_d = nc.declare_dram_parameter("acc_w", [P, nblk], fp32, isOutput=True)

    with TileContext(nc) as tc, ExitStack() as ctx:
        const = ctx.enter_context(tc.tile_pool(name="const", bufs=1))
        lhs_pool = ctx.enter_context(tc.tile_pool(name="lhs", bufs=1))
        rhs_pool = ctx.enter_context(tc.tile_pool(name="rhs", bufs=3))
        sc_pool = ctx.enter_context(tc.tile_pool(name="scratch", bufs=3))
        psA_pool = ctx.enter_context(tc.tile_pool(name="psA", bufs=2, space="PSUM"))
        psB_pool = ctx.enter_context(tc.tile_pool(name="psB", bufs=2, space="PSUM"))
        acc_pool = ctx.enter_context(tc.tile_pool(name="acc", bufs=1))

        ones2 = const.tile([2, P], bf16)
        nc.any.memset(ones2[:], 1.0)
        mbias = const.tile([P, 1], fp32)
        nc.any.memset(mbias[:], MARGIN)
        bias_t = const.tile([P, rt_n], fp32)
        nc.sync.dma_start(out=bias_t[:], in_=bias[:])
        labl_t = const.tile([10, rpc], bf16)
        nc.sync.dma_start(out=labl_t[:], in_=labl[:])
        lhs_t = lhs_pool.tile([P, kch, rpc], bf16)
        nc.sync.dma_start(out=lhs_t[:], in_=lhsT.rearrange("(c p) m -> p c m", p=P))

        acct_t = acc_pool.tile([P, nblk], fp32)
        accw_t = acc_pool.tile([P, nblk], fp32)

        qT_r = qT.rearrange("(c p) n -> p c n", p=P)

        for w in range(nwin_run):
            cs = slice(w * WINW, (w + 1) * WINW)
            rhs_t = rhs_pool.tile([P, kch, WINW], bf16, tag="rhs")
            nc.sync.dma_start(out=rhs_t[:], in_=qT_r[:, :, cs])
            sqf_t = rhs_pool.tile([2, WINW], bf16, tag="sqf")
            nc.sync.dma_start(out=sqf_t[:], in_=sqf[:, cs])
            labr_t = rhs_pool.tile([10, WINW], bf16, tag="labr")
            nc.sync.dma_start(out=labr_t[:], in_=labr[:, cs])
            for r in range(rt_n):
                blk = w * rt_n + r
                ms = slice(r * P, (r + 1) * P)
                psA = psA_pool.tile([P, WINW], fp32, tag="psA")
                psB = psB_pool.tile([P, WINW], fp32, tag="psB")
                for c in range(kch):
                    nc.tensor.matmul(
                        psA[:], lhs_t[:, c, ms], rhs_t[:, c, :],
                        start=(c == 0), stop=False,
                    )
                nc.tensor.matmul(psA[:], ones2[:], sqf_t[:], start=False, stop=True)
                nc.tensor.matmul(psB[:], labl_t[:, ms], labr_t[:], start=True, stop=True)

                d2 = sc_pool.tile([P, WINW], fp32, tag="d2")
                nc.scalar.activation(
                    d2[:], psA[:], Act.Relu,
                    bias=bias_t[:, r : r + 1], scale=-1.0 / dfeat,
                )
                t_ = sc_pool.tile([P, WINW], fp32, tag="t")
                nc.scalar.activation(
                    t_[:], d2[:], Act.Relu,
                    bias=mbias[:], scale=-1.0,
                    accum_out=acct_t[:, blk : blk + 1],
                )
                u_ = sc_pool.tile([P, WINW], fp32, tag="u")
                nc.vector.scalar_tensor_tensor(
                    u_[:], in0=t_[:], scalar=-1.0, in1=d2[:],
                    op0=Alu.mult, op1=Alu.add,
                )
                y_ = sc_pool.tile([P, WINW], fp32, tag="y")
                nc.vector.scalar_tensor_tensor(
                    y_[:], in0=psB[:], scalar=-LAM, in1=u_[:],
                    op0=Alu.mult, op1=Alu.add,
                )
                w_ = sc_pool.tile([P, WINW], fp32, tag="w")
                nc.vector.tensor_scalar(
                    w_[:], y_[:], 0.0, None, Alu.max, Alu.add,
                    accum_out=accw_t[:, blk : blk + 1],
                )

        nc.sync.dma_start(out=acct_d[:], in_=acct_t[:])
        nc.sync.dma_start(out=accw_d[:], in_=accw_t[:])
    nc.compile()
    return nc


def host_prep(predict, gt, b=B, dfeat=DFEAT, ncores=NCORES):
    """Build per-core input maps + the host-side label-pair count."""
    p = np.asarray(predict, np.float32).reshape(b, dfeat)
    q = p * np.float32(math.sqrt(2.0))
    qb = q.astype(BF16)
    qf = qb.astype(np.float32)
    stil = (0.5 * np.einsum("ij,ij->i", qf.astype(np.float64), qf.astype(np.float64))).astype(np.float32)
    smc = (stil - np.float32(C_OFF)).astype(np.float32)
    s_hi = smc.astype(BF16)
    s_lo = (smc - s_hi.astype(np.float32)).astype(BF16)
    sqf_full = np.stack([-s_hi, -s_lo]).astype(BF16)          # [2, B]

    g = np.asarray(gt).reshape(-1).astype(np.int64)
    da, db_, dc = g % 10, (g // 10) % 10, g // 100
    onesb = np.ones(b, np.float32)
    labr_full = np.stack(
        [onesb, da, da * da, onesb, db_, db_ * db_, onesb, dc, dc * dc, onesb]
    ).astype(BF16)                                            # [10, B]
    labl_full = np.stack(
        [da * da, -2.0 * da, onesb, db_ * db_, -2.0 * db_, onesb,
         dc * dc, -2.0 * dc, onesb, np.full(b, -2.0 / LAM, np.float32)]
    ).astype(BF16)                                            # [10, B]
    bias_full = ((stil + np.float32(C_OFF)) / np.float32(dfeat)).astype(np.float32)

    qT = np.ascontiguousarray(qb.T)                           # [D, B] bf16
    rpc = b // ncores
    rt_n = rpc // P
    in_maps = []
    for k in range(ncores):
        rs = slice(k * rpc, (k + 1) * rpc)
        in_maps.append({
            "qT": qT,
            "lhsT": np.ascontiguousarray(qT[:, rs]),
            "sqf": sqf_full,
            "labr": labr_full,
            "labl": np.ascontiguousarray(labl_full[:, rs]),
            "bias": np.ascontiguousarray(bias_full[rs].reshape(rt_n, P).T),
        })
    n_label = int((np.bincount(g) ** 2).sum())
    return in_maps, n_label


def finish(results, n_label, b=B):
    s = 0.0
    for r in results:
        s += float(r["acc_t"].astype(np.float64).sum())
        s += float(r["acc_w"].astype(np.float64).sum())
    loss = (s - 2.0 * n_label) / (float(b) * (b - 1))
    return np.float32(loss)


def kernel(predict, gt):
    global LAST_EXEC_NS, LAST_RESULTS
    from concourse.bass_utils import run_bass_kernel_spmd

    # fp8 DoubleRow main GEMM: ~2x the bf16 tensor throughput; squared-norm
    # debias keeps the quantization error bias-free (rel err ~3e-4).
    nc = build_tri_nc(use_fp8=True)
    in_maps, n_label, n_masked = host_prep_tri(predict, gt, use_fp8=True)
    res = run_bass_kernel_spmd(nc, in_maps, list(range(NCORES)))
    LAST_EXEC_NS = res.exec_time_ns
    LAST_RESULTS = res
    return finish_tri(res.results, n_label, n_masked)



# revision 10
# speedup vs baseline: 3.1754x; 1.0349x over previous
"""ContrastiveLoss Trainium2 kernel (8 NeuronCores, SPMD).

Math: reference computes, over p = predict.reshape(B, D):
    d2_ij  = max(||p_i||^2 + ||p_j||^2 - 2 p_i.p_j, 0) / D
    lab_ij = [g_i == g_j]
    term   = lab*d2 + (1-lab)*relu(M - d2),  M = 2
    loss   = 2 * sum_{j>=i} term / (B*(B-1))

Device formulation (full matrix, diagonal self-cancelling):
  q = bf16(sqrt(2)*p); stil_i = (q_i.q_i)/2  (host fp32)
  PSUM_A = q_i.q_j - (stil_j - C)            (bf16 GEMM K=2048 + K=2 feature GEMM)
  d2     = relu(-PSUM_A/D + (stil_i + C)/D)  == ||p^_i - p^_j||^2/D  >= 0 exactly
  t      = relu(M - d2)                      (scalar engine, accum_out -> col sums)
  PSUM_B = dg2 - 2/LAM  where dg2 = sum_digit (d_i - d_j)^2  (exact bf16 GEMM K=10)
  w      = relu((d2 - t) + 2 - LAM*dg2)      (vector engine, accum_out -> col sums)
  sum over full matrix:  S = sum t + sum w  satisfies
  loss   = (S - 2*N_label) / (B*(B-1)),  N_label = sum_c count_c^2 (ordered, incl diag)
Each core owns 1024 rows x all 8192 cols; host sums 8x2 [128,128] accumulators.
"""

import os
import math
from contextlib import ExitStack

import numpy as np
import ml_dtypes

BF16 = ml_dtypes.bfloat16

B = 8192
DFEAT = 2048
NCORES = 8
P = 128
WINW = 512
MARGIN = 2.0
C_OFF = 2048.0
LAM = 64.0

LAST_EXEC_NS = None
LAST_RESULTS = None


def tri_entries(ncores=NCORES, nblks=16):
    """Per-core list of (row_block, col_block) in 512-units: 2 diagonal entries
    first, then 15 plain upper-triangle entries. Uniform length for every core."""
    per_core = []
    for k in range(ncores):
        a, bb = k, nblks - 1 - k
        ents = [(a, a), (bb, bb)]
        ents += [(a, c) for c in range(a + 1, nblks)]
        ents += [(bb, c) for c in range(bb + 1, nblks)]
        assert len(ents) == nblks + 1
        per_core.append(ents)
    return per_core


def build_tri_nc(b=B, dfeat=DFEAT, ncores=NCORES, use_fp8=False, reps=1):
    """Triangle variant: 17 host-packed (lhs, rhs) stream entries per core,
    4 row-tiles each; entries 0,1 are diagonal blocks masked to j > i.
    use_fp8: main GEMM in float8e4 with DoubleRow (2 K-chunks per matmul).
    reps: repeat the whole body (timing builds; outputs are overwritten
    identically each rep)."""
    import concourse.bacc as bacc
    import concourse.mybir as mybir
    from concourse.tile import TileContext

    fp32 = mybir.dt.float32
    bf16 = mybir.dt.bfloat16
    qdt = mybir.dt.float8e4 if use_fp8 else bf16
    Act = mybir.ActivationFunctionType
    Alu = mybir.AluOpType

    kch = dfeat // P               # contraction chunks
    nent = (b // WINW) + 1         # 17 stream entries
    rt_n = WINW // P               # 4 row tiles per entry
    nblk = nent * rt_n             # 68 accumulator columns

    nc = bacc.Bacc()
    lhs_s = nc.declare_dram_parameter("lhs_s", [nent, dfeat, WINW], qdt, isOutput=False)
    rhs_s = nc.declare_dram_parameter("rhs_s", [nent, dfeat, WINW], qdt, isOutput=False)
    sqf_s = nc.declare_dram_parameter("sqf_s", [nent, 2, WINW], bf16, isOutput=False)
    labr_s = nc.declare_dram_parameter("labr_s", [nent, 10, WINW], bf16, isOutput=False)
    labl_s = nc.declare_dram_parameter("labl_s", [nent, 10, WINW], bf16, isOutput=False)
    bias_s = nc.declare_dram_parameter("bias_s", [nent, P, rt_n], fp32, isOutput=False)
    acct_d = nc.declare_dram_parameter("acc_t", [P, nblk], fp32, isOutput=True)
    accw_d = nc.declare_dram_parameter("acc_w", [P, nblk], fp32, isOutput=True)

    with TileContext(nc) as tc, ExitStack() as ctx:
        const = ctx.enter_context(tc.tile_pool(name="const", bufs=1))
        str_pool = ctx.enter_context(tc.tile_pool(name="streams", bufs=3))
        sc_pool = ctx.enter_context(tc.tile_pool(name="scratch", bufs=3))
        psA_pool = ctx.enter_context(tc.tile_pool(name="psA", bufs=2, space="PSUM"))
        psB_pool = ctx.enter_context(tc.tile_pool(name="psB", bufs=2, space="PSUM"))
        acc_pool = ctx.enter_context(tc.tile_pool(name="acc", bufs=1))

        ones2 = const.tile([2, P], bf16)
        nc.any.memset(ones2[:], 1.0)
        mbias = const.tile([P, 1], fp32)
        nc.any.memset(mbias[:], MARGIN)
        acct_t = acc_pool.tile([P, nblk], fp32)
        accw_t = acc_pool.tile([P, nblk], fp32)

        lhs_r = lhs_s.rearrange("e (c p) n -> e p c n", p=P)
        rhs_r = rhs_s.rearrange("e (c p) n -> e p c n", p=P)

        for e in [e for _ in range(reps) for e in range(nent)]:
            lhs_t = str_pool.tile([P, kch, WINW], qdt, tag="lhs")
            nc.sync.dma_start(out=lhs_t[:], in_=lhs_r[e])
            rhs_t = str_pool.tile([P, kch, WINW], qdt, tag="rhs")
            nc.sync.dma_start(out=rhs_t[:], in_=rhs_r[e])
            sqf_t = str_pool.tile([2, WINW], bf16, tag="sqf")
            nc.sync.dma_start(out=sqf_t[:], in_=sqf_s[e])
            labr_t = str_pool.tile([10, WINW], bf16, tag="labr")
            nc.sync.dma_start(out=labr_t[:], in_=labr_s[e])
            labl_t = str_pool.tile([10, WINW], bf16, tag="labl")
            nc.sync.dma_start(out=labl_t[:], in_=labl_s[e])
            bias_t = str_pool.tile([P, rt_n], fp32, tag="bias")
            nc.sync.dma_start(out=bias_t[:], in_=bias_s[e])
            for r in range(rt_n):
                blk = e * rt_n + r
                ms = slice(r * P, (r + 1) * P)
                psA = psA_pool.tile([P, WINW], fp32, tag="psA")
                psB = psB_pool.tile([P, WINW], fp32, tag="psB")
                if use_fp8:
                    for c in range(0, kch, 2):
                        nc.tensor.matmul(
                            psA[:], lhs_t[:, c : c + 2, ms], rhs_t[:, c : c + 2, :],
                            start=(c == 0), stop=False,
                            perf_mode=mybir.MatmulPerfMode.DoubleRow,
                        )
                else:
                    for c in range(kch):
                        nc.tensor.matmul(
                            psA[:], lhs_t[:, c, ms], rhs_t[:, c, :],
                            start=(c == 0), stop=False,
                        )
                nc.tensor.matmul(psA[:], ones2[:], sqf_t[:], start=False, stop=True)
                nc.tensor.matmul(psB[:], labl_t[:, ms], labr_t[:], start=True, stop=True)

                d2 = sc_pool.tile([P, WINW], fp32, tag="d2")
                nc.scalar.activation(
                    d2[:], psA[:], Act.Relu,
                    bias=bias_t[:, r : r + 1], scale=-1.0 / dfeat,
                )
                if e < 2:
                    # diagonal block: keep strictly-upper cells (col > row),
                    # zero the rest; host corrects the t=M per zeroed cell.
                    d2m = sc_pool.tile([P, WINW], fp32, tag="d2m")
                    nc.gpsimd.affine_select(
                        d2m[:], d2[:], pattern=[[1, WINW]],
                        compare_op=Alu.is_gt, fill=0.0,
                        base=-(r * P), channel_multiplier=-1,
                    )
                    d2 = d2m
                t_ = sc_pool.tile([P, WINW], fp32, tag="t")
                nc.scalar.activation(
                    t_[:], d2[:], Act.Relu,
                    bias=mbias[:], scale=-1.0,
                    accum_out=acct_t[:, blk : blk + 1],
                )
                u_ = sc_pool.tile([P, WINW], fp32, tag="u")
                nc.vector.scalar_tensor_tensor(
                    u_[:], in0=t_[:], scalar=-1.0, in1=d2[:],
                    op0=Alu.mult, op1=Alu.add,
                )
                y_ = sc_pool.tile([P, WINW], fp32, tag="y")
                nc.vector.scalar_tensor_tensor(
                    y_[:], in0=psB[:], scalar=-LAM, in1=u_[:],
                    op0=Alu.mult, op1=Alu.add,
                )
                w_ = sc_pool.tile([P, WINW], fp32, tag="w")
                nc.vector.tensor_scalar(
                    w_[:], y_[:], 0.0, None, Alu.max, Alu.add,
                    accum_out=accw_t[:, blk : blk + 1],
                )

        nc.sync.dma_start(out=acct_d[:], in_=acct_t[:])
        nc.sync.dma_start(out=accw_d[:], in_=accw_t[:])
    nc.compile()
    return nc


def host_prep_tri(predict, gt, b=B, dfeat=DFEAT, ncores=NCORES, use_fp8=False):
    """Per-core input maps for the triangle variant + host correction counts.

    use_fp8: quantize q to float8_e4m3 and debias the squared norms by the
    known quantization error energy, so E[d2_hat] == d2 (the raw fp8
    ||p^_i - p^_j||^2 overshoots by (||e_i||^2+||e_j||^2)/D otherwise)."""
    p = np.asarray(predict, np.float32).reshape(b, dfeat)
    q = p * np.float32(math.sqrt(2.0))
    qb = q.astype(ml_dtypes.float8_e4m3 if use_fp8 else BF16)
    qf = qb.astype(np.float32)
    if use_fp8:
        # true norms: cancels the row-common part of the quantization error
        # (E[q.e] != 0 for coarse RN grids); diagonal cells are masked anyway.
        stil = (0.5 * np.einsum("ij,ij->i", q.astype(np.float64), q.astype(np.float64))).astype(np.float32)
    else:
        stil = (0.5 * np.einsum("ij,ij->i", qf.astype(np.float64), qf.astype(np.float64))).astype(np.float32)
    smc = (stil - np.float32(C_OFF)).astype(np.float32)
    s_hi = smc.astype(BF16)
    s_lo = (smc - s_hi.astype(np.float32)).astype(BF16)
    sqf_full = np.stack([-s_hi, -s_lo]).astype(BF16)

    g = np.asarray(gt).reshape(-1).astype(np.int64)
    da, db_, dc = g % 10, (g // 10) % 10, g // 100
    onesb = np.ones(b, np.float32)
    labr_full = np.stack(
        [onesb, da, da * da, onesb, db_, db_ * db_, onesb, dc, dc * dc, onesb]
    ).astype(BF16)
    labl_full = np.stack(
        [da * da, -2.0 * da, onesb, db_ * db_, -2.0 * db_, onesb,
         dc * dc, -2.0 * dc, onesb, np.full(b, -2.0 / LAM, np.float32)]
    ).astype(BF16)
    bias_full = ((stil + np.float32(C_OFF)) / np.float32(dfeat)).astype(np.float32)
    qT = np.ascontiguousarray(qb.T)

    nblks = b // WINW
    del q, qf
    rt_n = WINW // P
    entries = tri_entries(ncores, nblks)
    in_maps = []
    for k in range(ncores):
        ents = entries[k]
        lhs_sx = np.stack([qT[:, rb * WINW:(rb + 1) * WINW] for rb, _ in ents])
        rhs_sx = np.stack([qT[:, cb * WINW:(cb + 1) * WINW] for _, cb in ents])
        sqf_sx = np.stack([sqf_full[:, cb * WINW:(cb + 1) * WINW] for _, cb in ents])
        labr_sx = np.stack([labr_full[:, cb * WINW:(cb + 1) * WINW] for _, cb in ents])
        labl_sx = np.stack([labl_full[:, rb * WINW:(rb + 1) * WINW] for rb, _ in ents])
        bias_sx = np.stack([
            np.ascontiguousarray(bias_full[rb * WINW:(rb + 1) * WINW].reshape(rt_n, P).T)
            for rb, _ in ents])
        in_maps.append({
            "lhs_s": np.ascontiguousarray(lhs_sx),
            "rhs_s": np.ascontiguousarray(rhs_sx),
            "sqf_s": np.ascontiguousarray(sqf_sx),
            "labr_s": np.ascontiguousarray(labr_sx),
            "labl_s": np.ascontiguousarray(labl_sx),
            "bias_s": np.ascontiguousarray(bias_sx),
        })
    n_label = int((np.bincount(g) ** 2).sum())
    n_masked = ncores * 2 * (WINW * (WINW + 1) // 2)   # j<=i cells zeroed per diag entry
    return in_maps, n_label, n_masked


def finish_tri(results, n_label, n_masked, b=B):
    s = 0.0
    for r in results:
        s += float(r["acc_t"].astype(np.float64).sum())
        s += float(r["acc_w"].astype(np.float64).sum())
    n_lab_strict = (n_label - b) // 2
    s_strict = s - MARGIN * n_masked - 2.0 * n_lab_strict
    loss = 2.0 * s_strict / (float(b) * (b - 1))
    return np.float32(loss)


def cyc_entries(k, nblks=16):
    """Cyclic-uniform block assignment for core k: row blocks (k, k+8).
    A-entries d=0..8 (col (k+d)%16), B-entries d=0..7 (col (k+8+d)%16).
    Every unordered block pair {r,c} is covered exactly once; entries 0 and
    9 are the diagonal blocks. Identical program shape for every core."""
    a, bb = k, k + nblks // 2
    ents = [(a, (a + d) % nblks) for d in range(nblks // 2 + 1)]
    ents += [(bb, (bb + d) % nblks) for d in range(nblks // 2)]
    return ents


def dedup_ldweights(nc):
    """Remove InstLdweights whose weights AP repeats the immediately
    preceding InstLdweights (same tensor/offset/pattern/perf_mode) with no
    other PE weight load in between. The PE weight registers persist across
    matmuls, so the repeated load is redundant. Only drops instructions with
    no sync updates and whose waits are a subset of the kept LDW's waits
    (identical tile => identical waits in practice; else keep)."""
    import concourse.mybir as mybir

    n_drop = 0
    for blk in nc.m.functions[0].blocks:
        insts = blk.instructions
        prev_key = None
        prev_wait_names = None
        keep = []
        for inst in insts:
            if isinstance(inst, mybir.InstLdweights):
                ap = inst.ins[0]
                key = (repr(ap), repr(inst.perf_mode))
                si = inst.sync_info
                waits = tuple(sorted(repr(w) for w in si.on_wait)) if si else ()
                upds = tuple(si.on_update) if si else ()
                if (key == prev_key and not upds
                        and set(waits) <= set(prev_wait_names or ())):
                    n_drop += 1
                    continue
                prev_key = key
                prev_wait_names = waits
            elif isinstance(inst, mybir.InstMatmult):
                pass  # matmuls don't disturb loaded weights
            elif inst.engine == mybir.EngineType.PE:
                prev_key = None
            keep.append(inst)
        if n_drop:
            insts[:] = keep
    return n_drop


def build_cyc_nc(b=B, dfeat=DFEAT, ncores=NCORES, use_fp8=True, group=1,
                 dedup=False, reps=1):
    """Cyclic-uniform variant: 2 resident lhs row-blocks per core, 17
    streamed rhs windows, contiguous per-partition DMA lines. Entries 0 and
    9 are diagonal (masked); the rest use a fused 2-scalar+2-vector
    elementwise pipeline with no d2 clamp (off-diagonal d2 ~ 2, never near
    0, so the clamp only ever mattered on masked diagonal cells).

    group>1: process `group` entries sharing one lhs block together so the
    c-loop can reuse the stationary weights across `group` matmuls; with
    dedup=True the redundant InstLdweights are stripped post-build."""
    import concourse.bacc as bacc
    import concourse.mybir as mybir
    from concourse.tile import TileContext

    fp32 = mybir.dt.float32
    bf16 = mybir.dt.bfloat16
    qdt = mybir.dt.float8e4 if use_fp8 else bf16
    Act = mybir.ActivationFunctionType
    Alu = mybir.AluOpType

    kch = dfeat // P               # 16 contraction chunks
    nent = (b // WINW) + 1         # 17 entries
    rt_n = WINW // P               # 4 row tiles
    nblk = nent * rt_n             # 68 accumulator columns

    nc = bacc.Bacc()
    lhs2 = nc.declare_dram_parameter("lhs2", [2, P, kch * WINW], qdt, isOutput=False)
    rhs_s = nc.declare_dram_parameter("rhs_s", [nent, P, kch * WINW], qdt, isOutput=False)
    sqf_s = nc.declare_dram_parameter("sqf_s", [nent, 2, WINW], bf16, isOutput=False)
    labr_s = nc.declare_dram_parameter("labr_s", [nent, 10, WINW], bf16, isOutput=False)
    labl2 = nc.declare_dram_parameter("labl2", [2, 10, WINW], bf16, isOutput=False)
    sbias2 = nc.declare_dram_parameter("sbias2", [2, P, rt_n], fp32, isOutput=False)
    tbias2 = nc.declare_dram_parameter("tbias2", [2, P, rt_n], fp32, isOutput=False)
    acct_d = nc.declare_dram_parameter("acc_t", [P, nblk], fp32, isOutput=True)
    accw_d = nc.declare_dram_parameter("acc_w", [P, nblk], fp32, isOutput=True)

    # entry -> lhs block (0=A rows k, 1=B rows k+8); diagonal entries: 0, 9
    ent_blk = [0] * 9 + [1] * 8
    diag_ents = (0, 9)

    with TileContext(nc) as tc, ExitStack() as ctx:
        const = ctx.enter_context(tc.tile_pool(name="const", bufs=1))
        str_pool = ctx.enter_context(tc.tile_pool(name="streams", bufs=max(3, group + 1)))
        sc_pool = ctx.enter_context(tc.tile_pool(name="scratch", bufs=3))
        psA_pool = ctx.enter_context(tc.tile_pool(name="psA", bufs=2, space="PSUM"))
        psB_pool = ctx.enter_context(tc.tile_pool(name="psB", bufs=2, space="PSUM"))
        acc_pool = ctx.enter_context(tc.tile_pool(name="acc", bufs=1))

        ones2 = const.tile([2, P], bf16)
        nc.any.memset(ones2[:], 1.0)
        mbias = const.tile([P, 1], fp32)
        nc.any.memset(mbias[:], MARGIN)
        lhs_r2 = lhs2.rearrange("b p (c n) -> b p c n", n=WINW)
        lhs_t, labl_t, sbias_t, tbias_t = [], [], [], []
        for bi in range(2):
            lt = const.tile([P, kch, WINW], qdt, name=f"lhs{bi}")
            nc.sync.dma_start(out=lt[:], in_=lhs_r2[bi])
            lhs_t.append(lt)
            ll = const.tile([10, WINW], bf16, name=f"labl{bi}")
            nc.sync.dma_start(out=ll[:], in_=labl2[bi])
            labl_t.append(ll)
            sb = const.tile([P, rt_n], fp32, name=f"sbias{bi}")
            nc.sync.dma_start(out=sb[:], in_=sbias2[bi])
            sbias_t.append(sb)
            tb = const.tile([P, rt_n], fp32, name=f"tbias{bi}")
            nc.sync.dma_start(out=tb[:], in_=tbias2[bi])
            tbias_t.append(tb)

        acct_t = acc_pool.tile([P, nblk], fp32)
        accw_t = acc_pool.tile([P, nblk], fp32)

        rhs_r = rhs_s.rearrange("e p (c n) -> e p c n", n=WINW)

        def do_entry_tail(e, r, psA, psB):
            """Elementwise pipeline for row-tile r of entry e (PSUMs ready)."""
            blk = e * rt_n + r
            bi = ent_blk[e]
            if e in diag_ents:
                d2 = sc_pool.tile([P, WINW], fp32, tag="d2")
                nc.scalar.activation(
                    d2[:], psA[:], Act.Relu,
                    bias=sbias_t[bi][:, r : r + 1], scale=-1.0 / dfeat,
                )
                d2m = sc_pool.tile([P, WINW], fp32, tag="d2m")
                nc.gpsimd.affine_select(
                    d2m[:], d2[:], pattern=[[1, WINW]],
                    compare_op=Alu.is_gt, fill=0.0,
                    base=-(r * P), channel_multiplier=-1,
                )
                t_ = sc_pool.tile([P, WINW], fp32, tag="t")
                nc.scalar.activation(
                    t_[:], d2m[:], Act.Relu,
                    bias=mbias[:], scale=-1.0,
                    accum_out=acct_t[:, blk : blk + 1],
                )
                u_ = sc_pool.tile([P, WINW], fp32, tag="u")
                nc.vector.scalar_tensor_tensor(
                    u_[:], in0=t_[:], scalar=-1.0, in1=d2m[:],
                    op0=Alu.mult, op1=Alu.add,
                )
                y_ = sc_pool.tile([P, WINW], fp32, tag="y")
                nc.vector.scalar_tensor_tensor(
                    y_[:], in0=psB[:], scalar=-LAM, in1=u_[:],
                    op0=Alu.mult, op1=Alu.add,
                )
                w_ = sc_pool.tile([P, WINW], fp32, tag="w")
                nc.vector.tensor_scalar(
                    w_[:], y_[:], 0.0, None, Alu.max, Alu.add,
                    accum_out=accw_t[:, blk : blk + 1],
                )
            else:
                # fused: t = relu(psA/D + tb); v = -psA/D - t;
                # y = -LAM*psB + v; w = relu(y + sb)
                t_ = sc_pool.tile([P, WINW], fp32, tag="t")
                nc.scalar.activation(
                    t_[:], psA[:], Act.Relu,
                    bias=tbias_t[bi][:, r : r + 1], scale=1.0 / dfeat,
                    accum_out=acct_t[:, blk : blk + 1],
                )
                v_ = sc_pool.tile([P, WINW], fp32, tag="u")
                nc.vector.scalar_tensor_tensor(
                    v_[:], in0=psA[:], scalar=-1.0 / dfeat, in1=t_[:],
                    op0=Alu.mult, op1=Alu.subtract,
                )
                y_ = sc_pool.tile([P, WINW], fp32, tag="y")
                nc.vector.scalar_tensor_tensor(
                    y_[:], in0=psB[:], scalar=-LAM, in1=v_[:],
                    op0=Alu.mult, op1=Alu.add,
                )
                w_ = sc_pool.tile([P, WINW], fp32, tag="w")
                nc.scalar.activation(
                    w_[:], y_[:], Act.Relu,
                    bias=sbias_t[bi][:, r : r + 1], scale=1.0,
                    accum_out=accw_t[:, blk : blk + 1],
                )

        def load_streams(e):
            rhs_t = str_pool.tile([P, kch, WINW], qdt, tag="rhs")
            nc.sync.dma_start(out=rhs_t[:], in_=rhs_r[e])
            sqf_t = str_pool.tile([2, WINW], bf16, tag="sqf")
            nc.sync.dma_start(out=sqf_t[:], in_=sqf_s[e])
            labr_t = str_pool.tile([10, WINW], bf16, tag="labr")
            nc.sync.dma_start(out=labr_t[:], in_=labr_s[e])
            return rhs_t, sqf_t, labr_t

        def small_mms(e, r, psA, psB, sqf_t, labr_t):
            bi = ent_blk[e]
            ms = slice(r * P, (r + 1) * P)
            nc.tensor.matmul(psA[:], ones2[:], sqf_t[:], start=False, stop=True)
            nc.tensor.matmul(psB[:], labl_t[bi][:, ms], labr_t[:], start=True, stop=True)

        # group entries by shared lhs block: A entries 0..8, B entries 9..16
        groups = []
        for base, n in ((0, 9), (9, 8)):
            ents = list(range(base, base + n))
            for i in range(0, n, group):
                groups.append(ents[i:i + group])

        for _ in range(reps):
            for g in groups:
                streams = {e: load_streams(e) for e in g}
                bi = ent_blk[g[0]]
                for r in range(rt_n):
                    ms = slice(r * P, (r + 1) * P)
                    psAs = {e: psA_pool.tile([P, WINW], fp32, tag=f"psA{i}",
                                             name=f"psA{i}")
                            for i, e in enumerate(g)}
                    if use_fp8:
                        for c in range(0, kch, 2):
                            for e in g:
                                nc.tensor.matmul(
                                    psAs[e][:], lhs_t[bi][:, c : c + 2, ms],
                                    streams[e][0][:, c : c + 2, :],
                                    start=(c == 0), stop=False,
                                    perf_mode=mybir.MatmulPerfMode.DoubleRow,
                                )
                    else:
                        for c in range(kch):
                            for e in g:
                                nc.tensor.matmul(
                                    psAs[e][:], lhs_t[bi][:, c, ms],
                                    streams[e][0][:, c, :],
                                    start=(c == 0), stop=False,
                                )
                    for e in g:
                        psB = psB_pool.tile([P, WINW], fp32, tag="psB")
                        small_mms(e, r, psAs[e], psB, streams[e][1], streams[e][2])
                        do_entry_tail(e, r, psAs[e], psB)

        nc.sync.dma_start(out=acct_d[:], in_=acct_t[:])
        nc.sync.dma_start(out=accw_d[:], in_=accw_t[:])
    if dedup:
        n = dedup_ldweights(nc)
        assert n > 0
    nc.compile()
    return nc


def host_prep_cyc(predict, gt, b=B, dfeat=DFEAT, ncores=NCORES, use_fp8=True):
    """Per-core input maps for the cyclic-uniform variant."""
    p = np.asarray(predict, np.float32).reshape(b, dfeat)
    q = p * np.float32(math.sqrt(2.0))
    qb = q.astype(ml_dtypes.float8_e4m3 if use_fp8 else BF16)
    if use_fp8:
        stil = (0.5 * np.einsum("ij,ij->i", q.astype(np.float64), q.astype(np.float64))).astype(np.float32)
    else:
        qf = qb.astype(np.float32)
        stil = (0.5 * np.einsum("ij,ij->i", qf.astype(np.float64), qf.astype(np.float64))).astype(np.float32)
    smc = (stil - np.float32(C_OFF)).astype(np.float32)
    s_hi = smc.astype(BF16)
    s_lo = (smc - s_hi.astype(np.float32)).astype(BF16)
    sqf_full = np.stack([-s_hi, -s_lo]).astype(BF16)

    g = np.asarray(gt).reshape(-1).astype(np.int64)
    da, db_, dc = g % 10, (g // 10) % 10, g // 100
    onesb = np.ones(b, np.float32)
    labr_full = np.stack(
        [onesb, da, da * da, onesb, db_, db_ * db_, onesb, dc, dc * dc, onesb]
    ).astype(BF16)
    labl_full = np.stack(
        [da * da, -2.0 * da, onesb, db_ * db_, -2.0 * db_, onesb,
         dc * dc, -2.0 * dc, onesb, np.full(b, -2.0 / LAM, np.float32)]
    ).astype(BF16)
    sbias_full = ((stil + np.float32(C_OFF)) / np.float32(dfeat)).astype(np.float32)
    tbias_full = (np.float32(MARGIN) - sbias_full).astype(np.float32)

    kch = dfeat // P
    rt_n = WINW // P
    # contiguous pack: [P, kch*WINW] per 512-row block, lines contiguous
    qT = np.ascontiguousarray(qb.T)                     # [D, B]
    nblks = b // WINW

    def pack_block(cb):
        blk = qT[:, cb * WINW:(cb + 1) * WINW]          # [D, W]
        return np.ascontiguousarray(
            blk.reshape(kch, P, WINW).transpose(1, 0, 2).reshape(P, kch * WINW))

    packed = [pack_block(cb) for cb in range(nblks)]

    in_maps = []
    for k in range(ncores):
        ents = cyc_entries(k, nblks)
        rA, rB = ents[0][0], ents[9][0]
        cols = [c for _, c in ents]
        lhs2 = np.stack([packed[rA], packed[rB]])
        rhs_sx = np.stack([packed[c] for c in cols])
        sqf_sx = np.stack([sqf_full[:, c * WINW:(c + 1) * WINW] for c in cols])
        labr_sx = np.stack([labr_full[:, c * WINW:(c + 1) * WINW] for c in cols])
        labl2_x = np.stack([labl_full[:, r * WINW:(r + 1) * WINW] for r in (rA, rB)])
        sb2 = np.stack([
            np.ascontiguousarray(sbias_full[r * WINW:(r + 1) * WINW].reshape(rt_n, P).T)
            for r in (rA, rB)])
        tb2 = np.stack([
            np.ascontiguousarray(tbias_full[r * WINW:(r + 1) * WINW].reshape(rt_n, P).T)
            for r in (rA, rB)])
        in_maps.append({
            "lhs2": np.ascontiguousarray(lhs2),
            "rhs_s": np.ascontiguousarray(rhs_sx),
            "sqf_s": np.ascontiguousarray(sqf_sx),
            "labr_s": np.ascontiguousarray(labr_sx),
            "labl2": np.ascontiguousarray(labl2_x),
            "sbias2": sb2,
            "tbias2": tb2,
        })
    n_label = int((np.bincount(g) ** 2).sum())
    n_masked = ncores * 2 * (WINW * (WINW + 1) // 2)
    return in_maps, n_label, n_masked


def build_nc(b=B, dfeat=DFEAT, ncores=NCORES, nwin_override=None):
    import concourse.bass as bass
    import concourse.bacc as bacc
    import concourse.mybir as mybir
    from concourse.tile import TileContext

    fp32 = mybir.dt.float32
    bf16 = mybir.dt.bfloat16
    Act = mybir.ActivationFunctionType
    Alu = mybir.AluOpType

    rpc = b // ncores              # rows per core
    rt_n = rpc // P                # row tiles per core
    nwin = b // WINW               # column windows
    nwin_run = nwin if nwin_override is None else nwin_override
    kch = dfeat // P               # contraction chunks
    nblk = nwin * rt_n

    nc = bacc.Bacc()
    qT = nc.declare_dram_parameter("qT", [dfeat, b], bf16, isOutput=False)
    lhsT = nc.declare_dram_parameter("lhsT", [dfeat, rpc], bf16, isOutput=False)
    sqf = nc.declare_dram_parameter("sqf", [2, b], bf16, isOutput=False)
    labr = nc.declare_dram_parameter("labr", [10, b], bf16, isOutput=False)
    labl = nc.declare_dram_parameter("labl", [10, rpc], bf16, isOutput=False)
    bias = nc.declare_dram_parameter("bias", [P, rt_n], fp32, isOutput=False)
    acct_d = nc.declare_dram_parameter("acc_t", [P, nblk], fp32, isOutput=True)
    accw_d = nc.declare_dram_parameter("acc_w", [P, nblk], fp32, isOutput=True)

    with TileContext(nc) as tc, ExitStack() as ctx:
        const = ctx.enter_context(tc.tile_pool(name="const", bufs=1))
        lhs_pool = ctx.enter_context(tc.tile_pool(name="lhs", bufs=1))
        rhs_pool = ctx.enter_context(tc.tile_pool(name="rhs", bufs=3))
        sc_pool = ctx.enter_context(tc.tile_pool(name="scratch", bufs=3))
        psA_pool = ctx.enter_context(tc.tile_pool(name="psA", bufs=2, space="PSUM"))
        psB_pool = ctx.enter_context(tc.tile_pool(name="psB", bufs=2, space="PSUM"))
        acc_pool = ctx.enter_context(tc.tile_pool(name="acc", bufs=1))

        ones2 = const.tile([2, P], bf16)
        nc.any.memset(ones2[:], 1.0)
        mbias = const.tile([P, 1], fp32)
        nc.any.memset(mbias[:], MARGIN)
        bias_t = const.tile([P, rt_n], fp32)
        nc.sync.dma_start(out=bias_t[:], in_=bias[:])
        labl_t = const.tile([10, rpc], bf16)
        nc.sync.dma_start(out=labl_t[:], in_=labl[:])
        lhs_t = lhs_pool.tile([P, kch, rpc], bf16)
        nc.sync.dma_start(out=lhs_t[:], in_=lhsT.rearrange("(c p) m -> p c m", p=P))

        acct_t = acc_pool.tile([P, nblk], fp32)
        accw_t = acc_pool.tile([P, nblk], fp32)

        qT_r = qT.rearrange("(c p) n -> p c n", p=P)

        for w in range(nwin_run):
            cs = slice(w * WINW, (w + 1) * WINW)
            rhs_t = rhs_pool.tile([P, kch, WINW], bf16, tag="rhs")
            nc.sync.dma_start(out=rhs_t[:], in_=qT_r[:, :, cs])
            sqf_t = rhs_pool.tile([2, WINW], bf16, tag="sqf")
            nc.sync.dma_start(out=sqf_t[:], in_=sqf[:, cs])
            labr_t = rhs_pool.tile([10, WINW], bf16, tag="labr")
            nc.sync.dma_start(out=labr_t[:], in_=labr[:, cs])
            for r in range(rt_n):
                blk = w * rt_n + r
                ms = slice(r * P, (r + 1) * P)
                psA = psA_pool.tile([P, WINW], fp32, tag="psA")
                psB = psB_pool.tile([P, WINW], fp32, tag="psB")
                for c in range(kch):
                    nc.tensor.matmul(
                        psA[:], lhs_t[:, c, ms], rhs_t[:, c, :],
                        start=(c == 0), stop=False,
                    )
                nc.tensor.matmul(psA[:], ones2[:], sqf_t[:], start=False, stop=True)
                nc.tensor.matmul(psB[:], labl_t[:, ms], labr_t[:], start=True, stop=True)

                d2 = sc_pool.tile([P, WINW], fp32, tag="d2")
                nc.scalar.activation(
                    d2[:], psA[:], Act.Relu,
                    bias=bias_t[:, r : r + 1], scale=-1.0 / dfeat,
                )
                t_ = sc_pool.tile([P, WINW], fp32, tag="t")
                nc.scalar.activation(
                    t_[:], d2[:], Act.Relu,
                    bias=mbias[:], scale=-1.0,
                    accum_out=acct_t[:, blk : blk + 1],
                )
                u_ = sc_pool.tile([P, WINW], fp32, tag="u")
                nc.vector.scalar_tensor_tensor(
                    u_[:], in0=t_[:], scalar=-1.0, in1=d2[:],
                    op0=Alu.mult, op1=Alu.add,
                )
                y_ = sc_pool.tile([P, WINW], fp32, tag="y")
                nc.vector.scalar_tensor_tensor(
                    y_[:], in0=psB[:], scalar=-LAM, in1=u_[:],
                    op0=Alu.mult, op1=Alu.add,
                )
                w_ = sc_pool.tile([P, WINW], fp32, tag="w")
                nc.vector.tensor_scalar(
                    w_[:], y_[:], 0.0, None, Alu.max, Alu.add,
                    accum_out=accw_t[:, blk : blk + 1],
                )

        nc.sync.dma_start(out=acct_d[:], in_=acct_t[:])
        nc.sync.dma_start(out=accw_d[:], in_=accw_t[:])
    nc.compile()
    return nc


def host_prep(predict, gt, b=B, dfeat=DFEAT, ncores=NCORES):
    """Build per-core input maps + the host-side label-pair count."""
    p = np.asarray(predict, np.float32).reshape(b, dfeat)
    q = p * np.float32(math.sqrt(2.0))
    qb = q.astype(BF16)
    qf = qb.astype(np.float32)
    stil = (0.5 * np.einsum("ij,ij->i", qf.astype(np.float64), qf.astype(np.float64))).astype(np.float32)
    smc = (stil - np.float32(C_OFF)).astype(np.float32)
    s_hi = smc.astype(BF16)
    s_lo = (smc - s_hi.astype(np.float32)).astype(BF16)
    sqf_full = np.stack([-s_hi, -s_lo]).astype(BF16)          # [2, B]

    g = np.asarray(gt).reshape(-1).astype(np.int64)
    da, db_, dc = g % 10, (g // 10) % 10, g // 100
    onesb = np.ones(b, np.float32)
    labr_full = np.stack(
        [onesb, da, da * da, onesb, db_, db_ * db_, onesb, dc, dc * dc, onesb]
    ).astype(BF16)                                            # [10, B]
    labl_full = np.stack(
        [da * da, -2.0 * da, onesb, db_ * db_, -2.0 * db_, onesb,
         dc * dc, -2.0 * dc, onesb, np.full(b, -2.0 / LAM, np.float32)]
    ).astype(BF16)                                            # [10, B]
    bias_full = ((stil + np.float32(C_OFF)) / np.float32(dfeat)).astype(np.float32)

    qT = np.ascontiguousarray(qb.T)                           # [D, B] bf16
    rpc = b // ncores
    rt_n = rpc // P
    in_maps = []
    for k in range(ncores):
        rs = slice(k * rpc, (k + 1) * rpc)
        in_maps.append({
            "qT": qT,
            "lhsT": np.ascontiguousarray(qT[:, rs]),
            "sqf": sqf_full,
            "labr": labr_full,
            "labl": np.ascontiguousarray(labl_full[:, rs]),
            "bias": np.ascontiguousarray(bias_full[rs].reshape(rt_n, P).T),
        })
    n_label = int((np.bincount(g) ** 2).sum())
    return in_maps, n_label


def finish(results, n_label, b=B):
    s = 0.0
    for r in results:
        s += float(r["acc_t"].astype(np.float64).sum())
        s += float(r["acc_w"].astype(np.float64).sum())
    loss = (s - 2.0 * n_label) / (float(b) * (b - 1))
    return np.float32(loss)


VARIANT = "cyc"  # "cyc" (resident lhs, grouped weights) or "tri" (fallback)


def build_best_nc():
    if VARIANT == "cyc":
        return build_cyc_nc(use_fp8=True, group=3, dedup=True)
    return build_tri_nc(use_fp8=True)


def kernel(predict, gt):
    global LAST_EXEC_NS, LAST_RESULTS
    from concourse.bass_utils import run_bass_kernel_spmd

    # fp8 DoubleRow main GEMM: ~2x the bf16 tensor throughput; squared-norm
    # debias keeps the quantization error bias-free (rel err ~3e-4).
    nc = build_best_nc()
    if VARIANT == "cyc":
        in_maps, n_label, n_masked = host_prep_cyc(predict, gt, use_fp8=True)
    else:
        in_maps, n_label, n_masked = host_prep_tri(predict, gt, use_fp8=True)
    res = run_bass_kernel_spmd(nc, in_maps, list(range(NCORES)))
    LAST_EXEC_NS = res.exec_time_ns
    LAST_RESULTS = res
    return finish_tri(res.results, n_label, n_masked)

